# revision 11
# baseline (speedup 1.0000x reference)
"""SuperGAT x15 Trainium2 kernel (8 NeuronCores, SPMD).

Self-contained: hardcodes all shapes. Strategy:
- Nodes permuted by "need" (balanced split slots), striped across 8 cores
  (core = rank % 8, pos = rank // 8). Each core owns 6250 nodes and all
  edges whose dst it owns.
- Per layer, each core holds a replicated DRAM table of rows
  [hp(32) bf16 | aL f32 | aR f32] = 36 bf16-slots = 72B at 256B stride.
- Messages gathered per edge-slot via dma_gather (int16 idxs) round-robin
  over 4 SWDGE queues with lookahead. The int16 range limit (32767) is
  handled with two overlapping table views: region A = rows [0, 32768),
  region B = rows [17232, 50000). Each node's in-edges are split between
  regions, padded to a UNIFORM per-chunk slot count D (SPMD: one program).
- Layout: node-per-partition, slots along free axis. Whole chunks (G
  blocks x D slots) are processed by single wide vector ops; segment
  softmax = free-axis reductions with 4D access patterns.
- Tails (h @ W_aug projection for the next layer) batched 4 blocks per
  matmul using block-diagonal weights.
- Per-layer exchange: own table rows -> DRAM bounce -> AllGather ->
  spread DMA into the 256B-stride gather table.
"""
import os
import hashlib
import numpy as np
import ml_dtypes

import concourse.bacc as bacc
import concourse.bass as bass
import concourse.tile as tile
from concourse import mybir, bass_utils, library_config
from concourse.masks import make_identity

dt = mybir.dt

# problem constants
N = 50000
E = 800000
D_IN = 128
H = 32
D_OUT = 16
L_FULL = 15
NEG = 0.2
NC = 8
NPC = N // NC            # 6250 nodes per core
NBLK = (NPC + 127) // 128  # 49 blocks
NPAD = NBLK * 128        # 6272 padded positions
T_HI = 32768
T_LO = N - T_HI          # 17232
ROWW = 36                # bf16 slots per table row (72B payload)
TABW = 128               # bf16 slots per table row stride (256B)

L_DEBUG = int(os.environ.get("SGAT_LAYERS", str(L_FULL)))
MAX_IDX_PER_GATHER = 16000
CHUNK_SLOTS = int(os.environ.get("SGAT_CHUNK", "120"))  # per-partition per-region
NSWQ = int(os.environ.get("SGAT_NSWQ", "4"))
LOOK = int(os.environ.get("SGAT_LOOK", "2"))


def _patch_dma_gather_assert():
    import inspect, textwrap
    if getattr(bass.BassGpSimd.dma_gather, "_sgat_patched", False):
        return
    src = inspect.getsource(bass.BassGpSimd.dma_gather)
    src = src.replace(
        "assert (\n            elem_size_bytes > 0 and elem_size_bytes % 256 == 0\n        )  # transpose restriction",
        "assert elem_size_bytes > 0")
    src = textwrap.dedent(src)
    ns = dict(bass.BassGpSimd.dma_gather.__globals__)
    exec(src, ns)
    fn = ns["dma_gather"]
    fn._sgat_patched = True
    bass.BassGpSimd.dma_gather = fn


_patch_dma_gather_assert()


# ----------------------------------------------------------------------------
# host-side graph preprocessing
# ----------------------------------------------------------------------------

def _preprocess(edge_index):
    src0 = edge_index[0].astype(np.int64)
    dst0 = edge_index[1].astype(np.int64)
    loops = np.arange(N, dtype=np.int64)
    src0 = np.concatenate([src0, loops])
    dst0 = np.concatenate([dst0, loops])

    deg = np.bincount(dst0, minlength=N)
    # Two-pass permutation: sort by degree, compute per-node "need"
    # (slots per region), re-sort by need so block maxima are tight.
    r = np.arange(N, dtype=np.int64)
    pid_of_rank = (r % NC) * NPC + r // NC

    def mk_perm(key):
        rank_of = np.argsort(-key, kind="stable")
        perm = np.empty(N, dtype=np.int64)
        perm[rank_of] = pid_of_rank
        return perm

    def calc_need(perm):
        psrc = perm[src0]
        pdst = perm[dst0]
        pdeg = np.bincount(pdst, minlength=N)
        nAf = np.bincount(pdst[psrc < T_LO], minlength=N)
        nBf = np.bincount(pdst[psrc >= T_HI], minlength=N)
        need = np.maximum(np.maximum(nAf, nBf), (pdeg + 1) // 2)
        return need, need[perm]

    perm = mk_perm(deg)
    _, need_orig = calc_need(perm)
    perm = mk_perm(need_orig)
    need, _ = calc_need(perm)
    inv_perm = np.empty(N, dtype=np.int64)
    inv_perm[perm] = np.arange(N, dtype=np.int64)

    psrc = perm[src0]
    pdst = perm[dst0]

    # block schedule: Dh[b] = max need over all cores' block b
    need_pad = np.zeros(NC * NPAD, dtype=np.int64)
    node_pid = np.arange(N)
    need_pad[(node_pid // NPC) * NPAD + node_pid % NPC] = need
    Dh = need_pad.reshape(NC, NBLK, 128).max(axis=(0, 2)).astype(np.int64)
    Dh = np.maximum(Dh, 1)

    # chunks of G consecutive blocks sharing a uniform slot count D
    chunks = []  # (b0, G, D, q0)
    offq = np.zeros(NBLK, dtype=np.int64)
    Dcap = np.zeros(NBLK, dtype=np.int64)
    b = 0
    q = 0
    while b < NBLK:
        d = int(Dh[b])
        g = 1
        while b + g < NBLK:
            nd = max(d, int(Dh[b + g]))
            if (g + 1) * nd > CHUNK_SLOTS or (g + 1) * nd * 128 > MAX_IDX_PER_GATHER:
                break
            d = nd
            g += 1
        chunks.append((b, g, d, q))
        for j in range(g):
            offq[b + j] = q + j * d
            Dcap[b + j] = d
        q += g * d
        b += g
    SA = int(q)

    # per-core slot tables
    eorder = np.lexsort((psrc, pdst))
    s_src = psrc[eorder]
    s_dst = pdst[eorder]
    starts = np.searchsorted(s_dst, np.arange(N))
    ends = np.searchsorted(s_dst, np.arange(N) + 1)

    idxA = np.zeros((NC, 128, SA), dtype=np.int16)
    idxB = np.zeros((NC, 128, SA), dtype=np.int16)
    maskA = np.full((NC, 128, SA), -1e30, dtype=np.float32)
    maskB = np.full((NC, 128, SA), -1e30, dtype=np.float32)

    for n in range(N):
        e0, e1 = starts[n], ends[n]
        if e0 == e1:
            continue
        ss = s_src[e0:e1]
        c = n // NPC
        p = n % NPC
        bb = p // 128
        pp = p % 128
        d = int(Dcap[bb])
        q0 = int(offq[bb])
        fa = ss[ss < T_LO]
        fb = ss[ss >= T_HI]
        fx = ss[(ss >= T_LO) & (ss < T_HI)]
        na, nb, nd = len(fa), len(fb), len(ss)
        lo_t = max(na, nd - d)
        hi_t = min(na + len(fx), d)
        ta = min(max((nd + 1) // 2, lo_t), hi_t)
        a_list = np.concatenate([fa, fx[: ta - na]])
        b_list = np.concatenate([fb, fx[ta - na:]])
        la, lb = len(a_list), len(b_list)
        assert la <= d and lb <= d, (n, la, lb, d)
        idxA[c, pp, q0:q0 + la] = a_list.astype(np.int16)
        maskA[c, pp, q0:q0 + la] = 0.0
        idxB[c, pp, q0:q0 + lb] = (b_list - T_LO).astype(np.int16)
        maskB[c, pp, q0:q0 + lb] = 0.0

    # wrap idxs for dma_gather: position i = q*128 + p -> [i%16, i//16], x8
    def wrap(idx):  # [128, SA] -> [128, SA*8] int16
        flat = idx.transpose(1, 0).reshape(-1)          # i-major
        w16 = flat.reshape(-1, 16).T                    # [16, SA*8]
        return np.tile(w16, (8, 1)).astype(np.int16)

    idxA_w = np.stack([wrap(idxA[c]) for c in range(NC)])
    idxB_w = np.stack([wrap(idxB[c]) for c in range(NC)])
    mask = np.stack([np.concatenate([maskA[c], maskB[c]], axis=1)
                     for c in range(NC)])               # [NC, 128, 2*SA]

    sched = dict(chunks=chunks, SA=SA)
    key = hashlib.sha256(
        (str(chunks) + str(L_DEBUG) + str(CHUNK_SLOTS) + str(NSWQ)
         + str(LOOK)).encode()).hexdigest()[:16]
    return dict(perm=perm, inv_perm=inv_perm, sched=sched, key=key,
                idxA=idxA_w, idxB=idxB_w, mask=mask)


# ----------------------------------------------------------------------------
# weights preprocessing
# ----------------------------------------------------------------------------

def _prep_weights(W0, b0, Ws, att_l, att_r, bs, W16, b16):
    # table_1 = (x @ W0 + b0) @ W1aug ; W1aug = [W1 | W1@al1 | W1@ar1]
    def aug(Wl, al, ar):
        A = np.zeros((H, ROWW), np.float32)
        A[:, :H] = Wl
        A[:, H] = Wl @ al
        A[:, H + 1] = Wl @ ar
        return A

    W1aug = aug(Ws[0], att_l[0], att_r[0])
    wfold = (W0 @ W1aug).astype(np.float32)            # [128, 36]
    bfold = (b0 @ W1aug).astype(np.float32)            # [36]

    # block-diagonal (4x) aug weights: wbd[li-1] is used by layer li's
    # tails to produce table_{li+1}; layer 15 uses w16bd instead.
    wbd = np.zeros((L_FULL, 128, 4 * ROWW), np.float32)
    for li in range(1, L_FULL):
        A = aug(Ws[li], att_l[li], att_r[li])
        for g in range(4):
            wbd[li - 1, g * H:(g + 1) * H, g * ROWW:(g + 1) * ROWW] = A
    w16bd = np.zeros((128, 4 * D_OUT), np.float32)
    for g in range(4):
        w16bd[g * H:(g + 1) * H, g * D_OUT:(g + 1) * D_OUT] = W16

    brep = np.tile(bs[:, None, :], (1, 128, 1)).astype(np.float32)
    bfold_rep = np.tile(bfold[None, :], (128, 1)).astype(np.float32)
    b16rep = np.tile(b16[None, :], (128, 1)).astype(np.float32)
    return dict(wfold=wfold, bfold=bfold_rep, wbd=wbd, w16bd=w16bd,
                brep=brep, b16rep=b16rep)


# ----------------------------------------------------------------------------
# program builder
# ----------------------------------------------------------------------------

def _build_program(sched):
    chunks = sched["chunks"]
    SA = sched["SA"]
    LN = L_DEBUG
    nch = len(chunks)

    nc = bacc.Bacc(num_devices=NC, num_swdge_queues=NSWQ)
    xT_in = nc.dram_tensor("xT", [D_IN, NPAD], dt.float32, kind="ExternalInput")
    idxA_in = nc.dram_tensor("idxA", [128, SA * 8], dt.int16, kind="ExternalInput")
    idxB_in = nc.dram_tensor("idxB", [128, SA * 8], dt.int16, kind="ExternalInput")
    mask_in = nc.dram_tensor("mask", [128, 2 * SA], dt.float32, kind="ExternalInput")
    wfold_in = nc.dram_tensor("wfold", [D_IN, ROWW], dt.float32, kind="ExternalInput")
    bfold_in = nc.dram_tensor("bfold", [128, ROWW], dt.float32, kind="ExternalInput")
    wbd_in = nc.dram_tensor("wbd", [L_FULL, 128, 4 * ROWW], dt.float32,
                            kind="ExternalInput")
    w16bd_in = nc.dram_tensor("w16bd", [128, 4 * D_OUT], dt.float32,
                              kind="ExternalInput")
    brep_in = nc.dram_tensor("brep", [L_FULL, 128, H], dt.float32,
                             kind="ExternalInput")
    b16_in = nc.dram_tensor("b16rep", [128, D_OUT], dt.float32,
                            kind="ExternalInput")

    if LN >= L_FULL:
        out_d = nc.dram_tensor("out", [NPAD, D_OUT], dt.float32,
                               kind="ExternalOutput")
    else:
        out_d = nc.dram_tensor("out", [NPAD, ROWW], dt.uint16,
                               kind="ExternalOutput")

    with tile.TileContext(nc) as tc:
        with tc.tile_pool(name="res", bufs=1) as res, \
             tc.tile_pool(name="gp", bufs=LOOK + 2) as gp, \
             tc.tile_pool(name="bp", bufs=2) as bp, \
             tc.tile_pool(name="wp", bufs=2) as wp, \
             tc.tile_pool(name="sp", bufs=2) as sp, \
             tc.tile_pool(name="tp", bufs=2) as tp, \
             tc.tile_pool(name="xp", bufs=2) as xp, \
             tc.tile_pool(name="pt", bufs=2, space="PSUM") as pt, \
             tc.tile_pool(name="pm", bufs=2, space="PSUM") as pm, \
             tc.tile_pool(name="dram", bufs=2, space="DRAM") as dram:

            nc.gpsimd.load_library(library_config.mlp)

            # residents
            idxA = res.tile([128, SA * 8], dt.int16)
            nc.sync.dma_start(out=idxA[:], in_=idxA_in[:])
            idxB = res.tile([128, SA * 8], dt.int16)
            nc.sync.dma_start(out=idxB[:], in_=idxB_in[:])
            maskr = res.tile([128, 2 * SA], dt.float32)
            nc.sync.dma_start(out=maskr[:], in_=mask_in[:])
            wfold = res.tile([D_IN, ROWW], dt.float32)
            nc.sync.dma_start(out=wfold[:], in_=wfold_in[:])
            bfold = res.tile([128, ROWW], dt.float32)
            nc.sync.dma_start(out=bfold[:], in_=bfold_in[:])
            wbd = res.tile([128, L_FULL * 4 * ROWW], dt.float32)
            nc.sync.dma_start(
                out=wbd[:].rearrange("p (l w) -> p l w", l=L_FULL),
                in_=wbd_in[:].rearrange("l p w -> p l w"))
            w16bd = res.tile([128, 4 * D_OUT], dt.float32)
            nc.sync.dma_start(out=w16bd[:], in_=w16bd_in[:])
            brep = res.tile([128, L_FULL * H], dt.float32)
            nc.sync.dma_start(
                out=brep[:].rearrange("p (l h) -> p l h", l=L_FULL),
                in_=brep_in[:].rearrange("l p h -> p l h"))
            b16r = res.tile([128, D_OUT], dt.float32)
            nc.sync.dma_start(out=b16r[:], in_=b16_in[:])
            ident = res.tile([128, 128], dt.float32)
            make_identity(nc, ident[:])

            own_tabs = [res.tile([128, NBLK, ROWW], dt.bfloat16, name=f"own{i}")
                        for i in range(2)]
            outstage = res.tile([128, NBLK, D_OUT], dt.float32)

            # ---------------- conv0 + fold into table_1 -----------------
            own = own_tabs[0]
            ownf0 = own[:].bitcast(dt.float32)
            for qd in range((NBLK + 3) // 4):
                b0 = qd * 4
                qw = min(4, NBLK - b0)
                xq = xp.tile([D_IN, qw * 128], dt.float32, tag="xq")
                nc.sync.dma_start(out=xq[:],
                                  in_=xT_in[:, b0 * 128:(b0 + qw) * 128])
                mmc = pm.tile([128, 4 * ROWW], dt.float32, space="PSUM",
                              tag="mm")
                for g in range(qw):
                    nc.tensor.matmul(out=mmc[:, g * ROWW:(g + 1) * ROWW],
                                     lhsT=xq[:, g * 128:(g + 1) * 128],
                                     rhs=wfold[:], start=True, stop=True)
                ps2 = sp.tile([128, 4 * ROWW], dt.float32, tag="c0add")
                nc.vector.tensor_tensor(
                    out=ps2[:, 0:qw * ROWW].rearrange("p (g w) -> p g w", g=qw),
                    in0=mmc[:, 0:qw * ROWW].rearrange("p (g w) -> p g w", g=qw),
                    in1=bfold[:].unsqueeze(1).broadcast_to([128, qw, ROWW]),
                    op=mybir.AluOpType.add)
                ps2v = ps2[:].rearrange("p (g w) -> p g w", g=4)
                nc.vector.tensor_copy(
                    out=own[:, b0:b0 + qw, 0:H],
                    in_=ps2v[:, 0:qw, 0:H])
                ps2f = ps2[:].rearrange("p (g w) -> p g w", g=4)
                nc.scalar.copy(
                    out=ownf0[:, b0:b0 + qw, H // 2:H // 2 + 2],
                    in_=ps2f[:, 0:qw, H:H + 2])

            def exchange(own_tab):
                bounce = dram.tile([NPAD, ROWW], dt.bfloat16, tag="bounce")
                nc.sync.dma_start(
                    out=bounce[:].rearrange("(b p) w -> p b w", p=128),
                    in_=own_tab[:])
                table = dram.tile([N, TABW], dt.bfloat16, tag="table")
                agout = dram.tile([N, ROWW], dt.bfloat16, tag="agout")
                nc.gpsimd.collective_compute(
                    "AllGather", mybir.AluOpType.bypass,
                    replica_groups=[list(range(NC))],
                    ins=[bounce[0:NPC, :]], outs=[agout[:]])
                nc.sync.dma_start(out=table[:, 0:ROWW], in_=agout[:])
                return table

            if LN == 0:
                nc.sync.dma_start(
                    out=out_d[:].rearrange("(b p) w -> p b w", p=128),
                    in_=own[:].bitcast(dt.uint16))
            table = exchange(own)

            gctr = [0]

            def do_gather(table, ci):
                b0, G, D, q0 = chunks[ci]
                csl = G * D
                gb = gp.tile([128, 2, csl, ROWW], dt.bfloat16, tag="gb")
                for rg in range(2):
                    tab_view = table[0:T_HI, 0:ROWW] if rg == 0 \
                        else table[T_LO:N, 0:ROWW]
                    idxr = idxA if rg == 0 else idxB
                    nidx = csl * 128
                    nc.gpsimd.dma_gather(
                        out_ap=gb[:, rg, :, :], in_ap=tab_view,
                        idxs_ap=idxr[:, q0 * 8:(q0 + csl) * 8],
                        num_idxs=nidx, num_idxs_reg=nidx,
                        elem_size=ROWW, elem_step=TABW,
                        single_packet=False,
                        queue_num=gctr[0] % NSWQ)
                    gctr[0] += 1
                return gb

            # ---------------- layers ----------------
            for li in range(1, LN + 1):
                own_prev = own_tabs[(li + 1) % 2]
                own_new = own_tabs[li % 2]
                ownf_prev = own_prev[:].bitcast(dt.float32)
                ownf_new = own_new[:].bitcast(dt.float32)
                last = (li == L_FULL)
                gbq = {}
                for j in range(min(LOOK + 1, nch)):
                    gbq[j] = do_gather(table, j)
                for ci in range(nch):
                    if ci + LOOK + 1 < nch:
                        gbq[ci + LOOK + 1] = do_gather(table, ci + LOOK + 1)
                    gb = gbq.pop(ci)
                    b0, G, D, q0 = chunks[ci]
                    csl = G * D
                    S2 = 2 * csl
                    gf32 = gb[:].bitcast(dt.float32)   # [128, 2, csl, 18]
                    hp_o = own_prev[:, b0:b0 + G, 0:H]
                    aR_o = ownf_prev[:, b0:b0 + G, H // 2 + 1]  # [p, G]

                    # pre-broadcast own rows across their D slots
                    hpb = bp.tile([128, csl, H], dt.bfloat16, tag="hpb")
                    nc.vector.tensor_copy(
                        out=hpb[:].rearrange("p (g d) f -> p g d f", g=G),
                        in_=hp_o.unsqueeze(2).broadcast_to([128, G, D, H]))
                    aRb = bp.tile([128, csl], dt.float32, tag="aRb")
                    nc.vector.tensor_copy(
                        out=aRb[:].rearrange("p (g d) -> p g d", g=G),
                        in_=aR_o.unsqueeze(2).broadcast_to([128, G, D]))

                    prod = wp.tile([128, 2, csl, H], dt.bfloat16, tag="prod")
                    nc.vector.tensor_tensor(
                        out=prod[:],
                        in0=gb[:, :, :, 0:H],
                        in1=hpb[:].unsqueeze(1).broadcast_to([128, 2, csl, H]),
                        op=mybir.AluOpType.mult)
                    logit = sp.tile([128, S2], dt.float32, tag="logit")
                    nc.vector.tensor_reduce(
                        out=logit[:],
                        in_=prod[:].rearrange("p r q f -> p (r q) f"),
                        axis=mybir.AxisListType.X, op=mybir.AluOpType.add)
                    sig = sp.tile([128, S2], dt.float32, tag="sig")
                    nc.scalar.activation(
                        out=sig[:], in_=logit[:],
                        func=mybir.ActivationFunctionType.Sigmoid)
                    alpha = sp.tile([128, S2], dt.float32, tag="alpha")
                    nc.vector.tensor_tensor(
                        out=alpha[:].rearrange("p (r q) -> p r q", r=2),
                        in0=gf32[:, :, :, H // 2],
                        in1=aRb[:].unsqueeze(1).broadcast_to([128, 2, csl]),
                        op=mybir.AluOpType.add)
                    nc.vector.tensor_tensor(out=alpha[:], in0=alpha[:],
                                            in1=sig[:],
                                            op=mybir.AluOpType.mult)
                    asc = sp.tile([128, S2], dt.float32, tag="asc")
                    nc.vector.tensor_scalar(
                        out=asc[:], in0=alpha[:], scalar1=NEG, scalar2=None,
                        op0=mybir.AluOpType.mult)
                    nc.vector.tensor_tensor(
                        out=alpha[:], in0=alpha[:], in1=asc[:],
                        op=mybir.AluOpType.max)
                    mk = maskr[:].rearrange("p (r s) -> p r s", r=2)[
                        :, :, q0:q0 + csl]
                    nc.vector.tensor_tensor(
                        out=alpha[:].rearrange("p (r q) -> p r q", r=2),
                        in0=alpha[:].rearrange("p (r q) -> p r q", r=2),
                        in1=mk, op=mybir.AluOpType.add)
                    am2 = sp.tile([128, 2 * G], dt.float32, tag="am2")
                    nc.vector.tensor_reduce(
                        out=am2[:],
                        in_=alpha[:].rearrange("p (rg d) -> p rg d", d=D),
                        axis=mybir.AxisListType.X, op=mybir.AluOpType.max)
                    nam = sp.tile([128, G], dt.float32, tag="nam")
                    nc.vector.tensor_reduce(
                        out=nam[:],
                        in_=am2[:].rearrange("p (r g) -> p g r", r=2),
                        axis=mybir.AxisListType.X, op=mybir.AluOpType.max,
                        negate=True)
                    nc.vector.tensor_tensor(
                        out=alpha[:].rearrange("p (r g d) -> p r g d",
                                               r=2, g=G),
                        in0=alpha[:].rearrange("p (r g d) -> p r g d",
                                               r=2, g=G),
                        in1=nam[:].unsqueeze(1).unsqueeze(3).broadcast_to(
                            [128, 2, G, D]),
                        op=mybir.AluOpType.add)
                    ex = sp.tile([128, S2], dt.float32, tag="ex")
                    nc.scalar.activation(
                        out=ex[:], in_=alpha[:],
                        func=mybir.ActivationFunctionType.Exp)
                    exb = sp.tile([128, S2], dt.bfloat16, tag="exb")
                    nc.vector.tensor_copy(out=exb[:], in_=ex[:])
                    den2 = sp.tile([128, 2 * G], dt.float32, tag="den2")
                    nc.vector.tensor_reduce(
                        out=den2[:],
                        in_=ex[:].rearrange("p (rg d) -> p rg d", d=D),
                        axis=mybir.AxisListType.X, op=mybir.AluOpType.add)
                    den1 = sp.tile([128, G], dt.float32, tag="den1")
                    nc.vector.tensor_reduce(
                        out=den1[:],
                        in_=den2[:].rearrange("p (r g) -> p g r", r=2),
                        axis=mybir.AxisListType.X, op=mybir.AluOpType.add)
                    rden = sp.tile([128, G], dt.float32, tag="rden")
                    nc.vector.reciprocal(out=rden[:], in_=den1[:])
                    # wv overwrites prod (hj*hi no longer needed)
                    nc.vector.tensor_tensor(
                        out=prod[:],
                        in0=gb[:, :, :, 0:H],
                        in1=exb[:].rearrange("p (r q) -> p r q", r=2)
                        .unsqueeze(3).broadcast_to([128, 2, csl, H]),
                        op=mybir.AluOpType.mult)
                    agg2 = tp.tile([128, 2, G, H], dt.float32, tag="agg2")
                    nc.vector.tensor_reduce(
                        out=agg2[:].rearrange("p r g f -> p (r g) f"),
                        in_=prod[:].rearrange("p r (g d) f -> p (r g) f d",
                                              g=G),
                        axis=mybir.AxisListType.X, op=mybir.AluOpType.add)
                    agg = tp.tile([128, G, H], dt.float32, tag="agg")
                    nc.vector.tensor_tensor(
                        out=agg[:], in0=agg2[:, 0], in1=agg2[:, 1],
                        op=mybir.AluOpType.add)
                    nc.vector.tensor_tensor(
                        out=agg[:], in0=agg[:],
                        in1=rden[:].unsqueeze(2).broadcast_to([128, G, H]),
                        op=mybir.AluOpType.mult)
                    nc.vector.tensor_tensor(
                        out=agg[:], in0=agg[:],
                        in1=brep[:].rearrange("p (l h) -> p l h",
                                              l=L_FULL)[:, li - 1]
                        .unsqueeze(1).broadcast_to([128, G, H]),
                        op=mybir.AluOpType.add)
                    hnext = tp.tile([128, G, H], dt.float32, tag="hnext")
                    nc.scalar.activation(
                        out=hnext[:], in_=agg[:],
                        func=mybir.ActivationFunctionType.Relu)

                    # tails: project 4 blocks at a time via block-diag weights
                    for t in range(0, G, 4):
                        gq = min(4, G - t)
                        tps = pt.tile([128, 128], dt.float32, space="PSUM",
                                      tag="hT")
                        nc.tensor.transpose(
                            out=tps[0:gq * H, :],
                            in_=hnext[:, t:t + gq, :].rearrange(
                                "p g f -> p (g f)"),
                            identity=ident[:])
                        hTs = sp.tile([128, 128], dt.float32, tag="hTs")
                        nc.scalar.copy(out=hTs[0:gq * H, :],
                                       in_=tps[0:gq * H, :])
                        mm = pm.tile([128, 4 * ROWW], dt.float32,
                                     space="PSUM", tag="mm")
                        if last:
                            nc.tensor.matmul(
                                out=mm[:, 0:gq * D_OUT],
                                lhsT=hTs[0:gq * H, :],
                                rhs=w16bd[0:gq * H, 0:gq * D_OUT],
                                start=True, stop=True)
                            nc.vector.tensor_tensor(
                                out=outstage[:, b0 + t:b0 + t + gq, :],
                                in0=mm[:, 0:gq * D_OUT].rearrange(
                                    "p (g w) -> p g w", g=gq),
                                in1=b16r[:].unsqueeze(1).broadcast_to(
                                    [128, gq, D_OUT]),
                                op=mybir.AluOpType.add)
                        else:
                            wslice = wbd[:].rearrange(
                                "p (l w) -> p l w", l=L_FULL)[:, li - 1, :]
                            nc.tensor.matmul(
                                out=mm[:, 0:gq * ROWW],
                                lhsT=hTs[0:gq * H, :],
                                rhs=wslice[0:gq * H, 0:gq * ROWW],
                                start=True, stop=True)
                            mmv = mm[:].rearrange("p (g w) -> p g w", g=4)
                            nc.vector.tensor_copy(
                                out=own_new[:, b0 + t:b0 + t + gq, 0:H],
                                in_=mmv[:, 0:gq, 0:H])
                            nc.scalar.copy(
                                out=ownf_new[:, b0 + t:b0 + t + gq,
                                             H // 2:H // 2 + 2],
                                in_=mmv[:, 0:gq, H:H + 2])
                if last:
                    nc.sync.dma_start(
                        out=out_d[:].rearrange("(b p) w -> p b w", p=128),
                        in_=outstage[:])
                elif li == LN:
                    nc.sync.dma_start(
                        out=out_d[:].rearrange("(b p) w -> p b w", p=128),
                        in_=own_new[:].bitcast(dt.uint16))
                else:
                    table = exchange(own_new)

    nc.compile()
    return nc


# ----------------------------------------------------------------------------
# entry point
# ----------------------------------------------------------------------------

_CACHE = {}
LAST_RES = None


def kernel(x, edge_index, W0, b0, Ws, att_l, att_r, bs, W16, b16):
    x = np.asarray(x, dtype=np.float32)
    edge_index = np.asarray(edge_index)
    pre = _preprocess(edge_index)
    wts = _prep_weights(np.asarray(W0, np.float32), np.asarray(b0, np.float32),
                        np.asarray(Ws, np.float32),
                        np.asarray(att_l, np.float32),
                        np.asarray(att_r, np.float32),
                        np.asarray(bs, np.float32),
                        np.asarray(W16, np.float32),
                        np.asarray(b16, np.float32))
    key = pre["key"]
    if key not in _CACHE:
        _CACHE[key] = _build_program(pre["sched"])
    nc = _CACHE[key]

    inv_perm = pre["inv_perm"]
    in_maps = []
    for c in range(NC):
        pids = np.arange(c * NPC, (c + 1) * NPC)
        orig = inv_perm[pids]
        xT = np.zeros((D_IN, NPAD), np.float32)
        xT[:, 0:NPC] = x[orig].T
        in_maps.append(dict(
            xT=xT, idxA=pre["idxA"][c], idxB=pre["idxB"][c],
            mask=pre["mask"][c].reshape(128, -1),
            wfold=wts["wfold"], bfold=wts["bfold"], wbd=wts["wbd"],
            w16bd=wts["w16bd"], brep=wts["brep"], b16rep=wts["b16rep"]))

    trace = bool(int(os.environ.get("SGAT_TRACE", "0")))
    tdir = os.environ.get("SGAT_TRACE_DIR") or None
    res = bass_utils.run_bass_kernel_spmd(nc, in_maps, core_ids=list(range(NC)),
                                          trace=trace, tmpdir=tdir)
    global LAST_RES
    LAST_RES = res

    if L_DEBUG >= L_FULL:
        out = np.zeros((N, D_OUT), np.float32)
        for c in range(NC):
            pids = np.arange(c * NPC, (c + 1) * NPC)
            out[inv_perm[pids]] = res.results[c]["out"][0:NPC]
        return out
    else:
        # debug: return raw table_{L+1} rows per permuted id
        out = np.zeros((N, ROWW), np.uint16)
        for c in range(NC):
            pids = np.arange(c * NPC, (c + 1) * NPC)
            out[inv_perm[pids]] = res.results[c]["out"][0:NPC]
        return out


# revision 15
# speedup vs baseline: 1.0995x; 1.0995x over previous
"""SuperGAT x15 Trainium2 kernel (8 NeuronCores, SPMD).

Self-contained: hardcodes all shapes. Strategy:
- Nodes permuted by "need" (balanced split slots), striped across 8 cores
  (core = rank % 8, pos = rank // 8). Each core owns 6250 nodes and all
  edges whose dst it owns.
- Per layer, each core holds a replicated DRAM table of rows
  [hp(32) bf16 | aL f32 | aR f32] = 36 bf16-slots = 72B at 256B stride.
- Messages gathered per edge-slot via dma_gather (int16 idxs) round-robin
  over 4 SWDGE queues with lookahead. The int16 range limit (32767) is
  handled with two overlapping table views: region A = rows [0, 32768),
  region B = rows [17232, 50000). Each node's in-edges are split between
  regions, padded to a UNIFORM per-chunk slot count D (SPMD: one program).
- Layout: node-per-partition, slots along free axis. Whole chunks (G
  blocks x D slots) are processed by single wide vector ops; segment
  softmax = free-axis reductions with 4D access patterns.
- Tails (h @ W_aug projection for the next layer) batched 4 blocks per
  matmul using block-diagonal weights.
- Per-layer exchange: own table rows -> DRAM bounce -> AllGather ->
  spread DMA into the 256B-stride gather table.
"""
import os
import hashlib
import numpy as np
import ml_dtypes

import concourse.bacc as bacc
import concourse.bass as bass
import concourse.tile as tile
from concourse import mybir, bass_utils, library_config
from concourse.masks import make_identity

dt = mybir.dt

# problem constants
N = 50000
E = 800000
D_IN = 128
H = 32
D_OUT = 16
L_FULL = 15
NEG = 0.2
NC = 8
NPC = N // NC            # 6250 nodes per core
NBLK = (NPC + 127) // 128  # 49 blocks
NPAD = NBLK * 128        # 6272 padded positions
T_HI = 32768
T_LO = N - T_HI          # 17232
ROWW = 36                # bf16 slots per table row (72B payload)
TABW = 128               # bf16 slots per table row stride (256B)

L_DEBUG = int(os.environ.get("SGAT_LAYERS", str(L_FULL)))
MAX_IDX_PER_GATHER = 16000
CHUNK_SLOTS = int(os.environ.get("SGAT_CHUNK", "120"))  # per-partition per-region
NSWQ = int(os.environ.get("SGAT_NSWQ", "4"))
LOOK = int(os.environ.get("SGAT_LOOK", "2"))


def _patch_dma_gather_assert():
    import inspect, textwrap
    if getattr(bass.BassGpSimd.dma_gather, "_sgat_patched", False):
        return
    src = inspect.getsource(bass.BassGpSimd.dma_gather)
    src = src.replace(
        "assert (\n            elem_size_bytes > 0 and elem_size_bytes % 256 == 0\n        )  # transpose restriction",
        "assert elem_size_bytes > 0")
    src = textwrap.dedent(src)
    ns = dict(bass.BassGpSimd.dma_gather.__globals__)
    exec(src, ns)
    fn = ns["dma_gather"]
    fn._sgat_patched = True
    bass.BassGpSimd.dma_gather = fn


_patch_dma_gather_assert()


# ----------------------------------------------------------------------------
# host-side graph preprocessing
# ----------------------------------------------------------------------------

def _preprocess(edge_index):
    src0 = edge_index[0].astype(np.int64)
    dst0 = edge_index[1].astype(np.int64)
    loops = np.arange(N, dtype=np.int64)
    src0 = np.concatenate([src0, loops])
    dst0 = np.concatenate([dst0, loops])

    deg = np.bincount(dst0, minlength=N)
    # Two-pass permutation: sort by degree, compute per-node "need"
    # (slots per region), re-sort by need so block maxima are tight.
    r = np.arange(N, dtype=np.int64)
    pid_of_rank = (r % NC) * NPC + r // NC

    def mk_perm(key):
        rank_of = np.argsort(-key, kind="stable")
        perm = np.empty(N, dtype=np.int64)
        perm[rank_of] = pid_of_rank
        return perm

    def calc_need(perm):
        psrc = perm[src0]
        pdst = perm[dst0]
        pdeg = np.bincount(pdst, minlength=N)
        nAf = np.bincount(pdst[psrc < T_LO], minlength=N)
        nBf = np.bincount(pdst[psrc >= T_HI], minlength=N)
        need = np.maximum(np.maximum(nAf, nBf), (pdeg + 1) // 2)
        return need, need[perm]

    perm = mk_perm(deg)
    _, need_orig = calc_need(perm)
    perm = mk_perm(need_orig)
    need, _ = calc_need(perm)
    inv_perm = np.empty(N, dtype=np.int64)
    inv_perm[perm] = np.arange(N, dtype=np.int64)

    psrc = perm[src0]
    pdst = perm[dst0]

    # block schedule: Dh[b] = max need over all cores' block b
    need_pad = np.zeros(NC * NPAD, dtype=np.int64)
    node_pid = np.arange(N)
    need_pad[(node_pid // NPC) * NPAD + node_pid % NPC] = need
    Dh = need_pad.reshape(NC, NBLK, 128).max(axis=(0, 2)).astype(np.int64)
    Dh = np.maximum(Dh, 1)

    # chunks of G consecutive blocks sharing a uniform slot count D.
    # First chunks are kept small so the first gather of each layer drains
    # quickly and vector work starts early (pipeline warm-up).
    chunks = []  # (b0, G, D, q0)
    offq = np.zeros(NBLK, dtype=np.int64)
    Dcap = np.zeros(NBLK, dtype=np.int64)
    b = 0
    q = 0
    while b < NBLK:
        cap = (40 if len(chunks) == 0 else
               64 if len(chunks) == 1 else CHUNK_SLOTS)
        d = int(Dh[b])
        g = 1
        while b + g < NBLK:
            nd = max(d, int(Dh[b + g]))
            if (g + 1) * nd > cap or (g + 1) * nd * 128 > MAX_IDX_PER_GATHER:
                break
            d = nd
            g += 1
        chunks.append((b, g, d, q))
        for j in range(g):
            offq[b + j] = q + j * d
            Dcap[b + j] = d
        q += g * d
        b += g
    SA = int(q)

    # per-core slot tables
    eorder = np.lexsort((psrc, pdst))
    s_src = psrc[eorder]
    s_dst = pdst[eorder]
    starts = np.searchsorted(s_dst, np.arange(N))
    ends = np.searchsorted(s_dst, np.arange(N) + 1)

    idxA = np.zeros((NC, 128, SA), dtype=np.int16)
    idxB = np.zeros((NC, 128, SA), dtype=np.int16)
    maskA = np.full((NC, 128, SA), -1e30, dtype=np.float32)
    maskB = np.full((NC, 128, SA), -1e30, dtype=np.float32)

    for n in range(N):
        e0, e1 = starts[n], ends[n]
        if e0 == e1:
            continue
        ss = s_src[e0:e1]
        c = n // NPC
        p = n % NPC
        bb = p // 128
        pp = p % 128
        d = int(Dcap[bb])
        q0 = int(offq[bb])
        fa = ss[ss < T_LO]
        fb = ss[ss >= T_HI]
        fx = ss[(ss >= T_LO) & (ss < T_HI)]
        na, nb, nd = len(fa), len(fb), len(ss)
        lo_t = max(na, nd - d)
        hi_t = min(na + len(fx), d)
        ta = min(max((nd + 1) // 2, lo_t), hi_t)
        a_list = np.concatenate([fa, fx[: ta - na]])
        b_list = np.concatenate([fb, fx[ta - na:]])
        la, lb = len(a_list), len(b_list)
        assert la <= d and lb <= d, (n, la, lb, d)
        idxA[c, pp, q0:q0 + la] = a_list.astype(np.int16)
        maskA[c, pp, q0:q0 + la] = 0.0
        idxB[c, pp, q0:q0 + lb] = (b_list - T_LO).astype(np.int16)
        maskB[c, pp, q0:q0 + lb] = 0.0

    # wrap idxs for dma_gather: position i = q*128 + p -> [i%16, i//16], x8
    def wrap(idx):  # [128, SA] -> [128, SA*8] int16
        flat = idx.transpose(1, 0).reshape(-1)          # i-major
        w16 = flat.reshape(-1, 16).T                    # [16, SA*8]
        return np.tile(w16, (8, 1)).astype(np.int16)

    idxA_w = np.stack([wrap(idxA[c]) for c in range(NC)])
    idxB_w = np.stack([wrap(idxB[c]) for c in range(NC)])
    mask = np.stack([np.concatenate([maskA[c], maskB[c]], axis=1)
                     for c in range(NC)])               # [NC, 128, 2*SA]

    sched = dict(chunks=chunks, SA=SA)
    key = hashlib.sha256(
        (str(chunks) + str(L_DEBUG) + str(CHUNK_SLOTS) + str(NSWQ)
         + str(LOOK)).encode()).hexdigest()[:16]
    return dict(perm=perm, inv_perm=inv_perm, sched=sched, key=key,
                idxA=idxA_w, idxB=idxB_w, mask=mask)


# ----------------------------------------------------------------------------
# weights preprocessing
# ----------------------------------------------------------------------------

def _prep_weights(W0, b0, Ws, att_l, att_r, bs, W16, b16):
    # table_1 = (x @ W0 + b0) @ W1aug ; W1aug = [W1 | W1@al1 | W1@ar1]
    def aug(Wl, al, ar):
        A = np.zeros((H, ROWW), np.float32)
        A[:, :H] = Wl
        A[:, H] = Wl @ al
        A[:, H + 1] = Wl @ ar
        return A

    W1aug = aug(Ws[0], att_l[0], att_r[0])
    wfold = (W0 @ W1aug).astype(np.float32)            # [128, 36]
    bfold = (b0 @ W1aug).astype(np.float32)            # [36]

    # block-diagonal (4x) aug weights: wbd[li-1] is used by layer li's
    # tails to produce table_{li+1}; layer 15 uses w16bd instead.
    wbd = np.zeros((L_FULL, 128, 4 * ROWW), np.float32)
    for li in range(1, L_FULL):
        A = aug(Ws[li], att_l[li], att_r[li])
        for g in range(4):
            wbd[li - 1, g * H:(g + 1) * H, g * ROWW:(g + 1) * ROWW] = A
    w16bd = np.zeros((128, 4 * D_OUT), np.float32)
    for g in range(4):
        w16bd[g * H:(g + 1) * H, g * D_OUT:(g + 1) * D_OUT] = W16

    brep = np.tile(bs[:, None, :], (1, 128, 1)).astype(np.float32)
    bfold_rep = np.tile(bfold[None, :], (128, 1)).astype(np.float32)
    b16rep = np.tile(b16[None, :], (128, 1)).astype(np.float32)
    return dict(wfold=wfold, bfold=bfold_rep, wbd=wbd, w16bd=w16bd,
                brep=brep, b16rep=b16rep)


# ----------------------------------------------------------------------------
# program builder
# ----------------------------------------------------------------------------

def _build_program(sched):
    chunks = sched["chunks"]
    SA = sched["SA"]
    LN = L_DEBUG
    nch = len(chunks)

    nc = bacc.Bacc(num_devices=NC, num_swdge_queues=NSWQ)
    xT_in = nc.dram_tensor("xT", [D_IN, NPAD], dt.float32, kind="ExternalInput")
    idxA_in = nc.dram_tensor("idxA", [128, SA * 8], dt.int16, kind="ExternalInput")
    idxB_in = nc.dram_tensor("idxB", [128, SA * 8], dt.int16, kind="ExternalInput")
    mask_in = nc.dram_tensor("mask", [128, 2 * SA], dt.float32, kind="ExternalInput")
    wfold_in = nc.dram_tensor("wfold", [D_IN, ROWW], dt.float32, kind="ExternalInput")
    bfold_in = nc.dram_tensor("bfold", [128, ROWW], dt.float32, kind="ExternalInput")
    wbd_in = nc.dram_tensor("wbd", [L_FULL, 128, 4 * ROWW], dt.float32,
                            kind="ExternalInput")
    w16bd_in = nc.dram_tensor("w16bd", [128, 4 * D_OUT], dt.float32,
                              kind="ExternalInput")
    brep_in = nc.dram_tensor("brep", [L_FULL, 128, H], dt.float32,
                             kind="ExternalInput")
    b16_in = nc.dram_tensor("b16rep", [128, D_OUT], dt.float32,
                            kind="ExternalInput")

    if LN >= L_FULL:
        out_d = nc.dram_tensor("out", [NPAD, D_OUT], dt.float32,
                               kind="ExternalOutput")
    else:
        out_d = nc.dram_tensor("out", [NPAD, ROWW], dt.uint16,
                               kind="ExternalOutput")

    with tile.TileContext(nc) as tc:
        with tc.tile_pool(name="res", bufs=1) as res, \
             tc.tile_pool(name="gp", bufs=LOOK + 2) as gp, \
             tc.tile_pool(name="bp", bufs=2) as bp, \
             tc.tile_pool(name="wp", bufs=2) as wp, \
             tc.tile_pool(name="sp", bufs=2) as sp, \
             tc.tile_pool(name="tp", bufs=2) as tp, \
             tc.tile_pool(name="xp", bufs=2) as xp, \
             tc.tile_pool(name="pt", bufs=2, space="PSUM") as pt, \
             tc.tile_pool(name="pm", bufs=2, space="PSUM") as pm, \
             tc.tile_pool(name="dram", bufs=2, space="DRAM") as dram:

            nc.gpsimd.load_library(library_config.mlp)

            # residents
            idxA = res.tile([128, SA * 8], dt.int16)
            nc.sync.dma_start(out=idxA[:], in_=idxA_in[:])
            idxB = res.tile([128, SA * 8], dt.int16)
            nc.sync.dma_start(out=idxB[:], in_=idxB_in[:])
            maskr = res.tile([128, 2 * SA], dt.float32)
            nc.sync.dma_start(out=maskr[:], in_=mask_in[:])
            wfold = res.tile([D_IN, ROWW], dt.float32)
            nc.sync.dma_start(out=wfold[:], in_=wfold_in[:])
            bfold = res.tile([128, ROWW], dt.float32)
            nc.sync.dma_start(out=bfold[:], in_=bfold_in[:])
            wbd = res.tile([128, L_FULL * 4 * ROWW], dt.float32)
            nc.sync.dma_start(
                out=wbd[:].rearrange("p (l w) -> p l w", l=L_FULL),
                in_=wbd_in[:].rearrange("l p w -> p l w"))
            w16bd = res.tile([128, 4 * D_OUT], dt.float32)
            nc.sync.dma_start(out=w16bd[:], in_=w16bd_in[:])
            brep = res.tile([128, L_FULL * H], dt.float32)
            nc.sync.dma_start(
                out=brep[:].rearrange("p (l h) -> p l h", l=L_FULL),
                in_=brep_in[:].rearrange("l p h -> p l h"))
            b16r = res.tile([128, D_OUT], dt.float32)
            nc.sync.dma_start(out=b16r[:], in_=b16_in[:])
            ident = res.tile([128, 128], dt.float32)
            make_identity(nc, ident[:])

            own_tabs = [res.tile([128, NBLK, ROWW], dt.bfloat16, name=f"own{i}")
                        for i in range(2)]
            outstage = res.tile([128, NBLK, D_OUT], dt.float32)

            # ---------------- conv0 + fold into table_1 -----------------
            own = own_tabs[0]
            ownf0 = own[:].bitcast(dt.float32)
            for qd in range((NBLK + 3) // 4):
                b0 = qd * 4
                qw = min(4, NBLK - b0)
                xq = xp.tile([D_IN, qw * 128], dt.float32, tag="xq")
                nc.sync.dma_start(out=xq[:],
                                  in_=xT_in[:, b0 * 128:(b0 + qw) * 128])
                mmc = pm.tile([128, 4 * ROWW], dt.float32, space="PSUM",
                              tag="mm")
                for g in range(qw):
                    nc.tensor.matmul(out=mmc[:, g * ROWW:(g + 1) * ROWW],
                                     lhsT=xq[:, g * 128:(g + 1) * 128],
                                     rhs=wfold[:], start=True, stop=True)
                ps2 = sp.tile([128, 4 * ROWW], dt.float32, tag="c0add")
                nc.vector.tensor_tensor(
                    out=ps2[:, 0:qw * ROWW].rearrange("p (g w) -> p g w", g=qw),
                    in0=mmc[:, 0:qw * ROWW].rearrange("p (g w) -> p g w", g=qw),
                    in1=bfold[:].unsqueeze(1).broadcast_to([128, qw, ROWW]),
                    op=mybir.AluOpType.add)
                ps2v = ps2[:].rearrange("p (g w) -> p g w", g=4)
                nc.vector.tensor_copy(
                    out=own[:, b0:b0 + qw, 0:H],
                    in_=ps2v[:, 0:qw, 0:H])
                ps2f = ps2[:].rearrange("p (g w) -> p g w", g=4)
                nc.scalar.copy(
                    out=ownf0[:, b0:b0 + qw, H // 2:H // 2 + 2],
                    in_=ps2f[:, 0:qw, H:H + 2])

            def exchange(own_tab):
                bounce = dram.tile([NPAD, ROWW], dt.bfloat16, tag="bounce")
                nc.sync.dma_start(
                    out=bounce[:].rearrange("(b p) w -> p b w", p=128),
                    in_=own_tab[:])
                table = dram.tile([N, TABW], dt.bfloat16, tag="table")
                if os.environ.get("SGAT_SPREAD", "1") == "1":
                    agout = dram.tile([N, ROWW], dt.bfloat16, tag="agout")
                    nc.gpsimd.collective_compute(
                        "AllGather", mybir.AluOpType.bypass,
                        replica_groups=[list(range(NC))],
                        ins=[bounce[0:NPC, :]], outs=[agout[:]])
                    # split the row-spread across both HWDGE rings
                    nh = N // 2
                    nc.sync.dma_start(out=table[0:nh, 0:ROWW],
                                      in_=agout[0:nh, :])
                    nc.scalar.dma_start(out=table[nh:N, 0:ROWW],
                                        in_=agout[nh:N, :])
                elif False:
                    nc.gpsimd.collective_compute(
                        "AllGather", mybir.AluOpType.bypass,
                        replica_groups=[list(range(NC))],
                        ins=[bounce[0:NPC, :]], outs=[table[:, 0:ROWW]])
                return table

            if LN == 0:
                nc.sync.dma_start(
                    out=out_d[:].rearrange("(b p) w -> p b w", p=128),
                    in_=own[:].bitcast(dt.uint16))
            table = exchange(own)

            gctr = [0]

            def do_gather(table, ci):
                b0, G, D, q0 = chunks[ci]
                csl = G * D
                gb = gp.tile([128, 2, csl, ROWW], dt.bfloat16, tag="gb")
                for rg in range(2):
                    tab_view = table[0:T_HI, 0:ROWW] if rg == 0 \
                        else table[T_LO:N, 0:ROWW]
                    idxr = idxA if rg == 0 else idxB
                    nidx = csl * 128
                    nc.gpsimd.dma_gather(
                        out_ap=gb[:, rg, :, :], in_ap=tab_view,
                        idxs_ap=idxr[:, q0 * 8:(q0 + csl) * 8],
                        num_idxs=nidx, num_idxs_reg=nidx,
                        elem_size=ROWW, elem_step=TABW,
                        single_packet=False,
                        queue_num=gctr[0] % NSWQ)
                    gctr[0] += 1
                return gb

            # ---------------- layers ----------------
            for li in range(1, LN + 1):
                own_prev = own_tabs[(li + 1) % 2]
                own_new = own_tabs[li % 2]
                ownf_prev = own_prev[:].bitcast(dt.float32)
                ownf_new = own_new[:].bitcast(dt.float32)
                last = (li == L_FULL)
                gbq = {}
                for j in range(min(LOOK + 1, nch)):
                    gbq[j] = do_gather(table, j)
                for ci in range(nch):
                    if ci + LOOK + 1 < nch:
                        gbq[ci + LOOK + 1] = do_gather(table, ci + LOOK + 1)
                    gb = gbq.pop(ci)
                    b0, G, D, q0 = chunks[ci]
                    csl = G * D
                    S2 = 2 * csl
                    gf32 = gb[:].bitcast(dt.float32)   # [128, 2, csl, 18]
                    hp_o = own_prev[:, b0:b0 + G, 0:H]
                    aR_o = ownf_prev[:, b0:b0 + G, H // 2 + 1]  # [p, G]

                    # pre-broadcast own rows across their D slots
                    hpb = bp.tile([128, csl, H], dt.bfloat16, tag="hpb")
                    nc.vector.tensor_copy(
                        out=hpb[:].rearrange("p (g d) f -> p g d f", g=G),
                        in_=hp_o.unsqueeze(2).broadcast_to([128, G, D, H]))
                    aRb = bp.tile([128, csl], dt.float32, tag="aRb")
                    nc.vector.tensor_copy(
                        out=aRb[:].rearrange("p (g d) -> p g d", g=G),
                        in_=aR_o.unsqueeze(2).broadcast_to([128, G, D]))

                    prod = wp.tile([128, 2, csl, H], dt.bfloat16, tag="prod")
                    nc.vector.tensor_tensor(
                        out=prod[:],
                        in0=gb[:, :, :, 0:H],
                        in1=hpb[:].unsqueeze(1).broadcast_to([128, 2, csl, H]),
                        op=mybir.AluOpType.mult)
                    logit = sp.tile([128, S2], dt.float32, tag="logit")
                    nc.vector.tensor_reduce(
                        out=logit[:],
                        in_=prod[:].rearrange("p r q f -> p (r q) f"),
                        axis=mybir.AxisListType.X, op=mybir.AluOpType.add)
                    sig = sp.tile([128, S2], dt.float32, tag="sig")
                    nc.scalar.activation(
                        out=sig[:], in_=logit[:],
                        func=mybir.ActivationFunctionType.Sigmoid)
                    alpha = sp.tile([128, S2], dt.float32, tag="alpha")
                    nc.vector.tensor_tensor(
                        out=alpha[:].rearrange("p (r q) -> p r q", r=2),
                        in0=gf32[:, :, :, H // 2],
                        in1=aRb[:].unsqueeze(1).broadcast_to([128, 2, csl]),
                        op=mybir.AluOpType.add)
                    nc.vector.tensor_tensor(out=alpha[:], in0=alpha[:],
                                            in1=sig[:],
                                            op=mybir.AluOpType.mult)
                    asc = sp.tile([128, S2], dt.float32, tag="asc")
                    nc.vector.tensor_scalar(
                        out=asc[:], in0=alpha[:], scalar1=NEG, scalar2=None,
                        op0=mybir.AluOpType.mult)
                    nc.vector.tensor_tensor(
                        out=alpha[:], in0=alpha[:], in1=asc[:],
                        op=mybir.AluOpType.max)
                    mk = maskr[:].rearrange("p (r s) -> p r s", r=2)[
                        :, :, q0:q0 + csl]
                    nc.vector.tensor_tensor(
                        out=alpha[:].rearrange("p (r q) -> p r q", r=2),
                        in0=alpha[:].rearrange("p (r q) -> p r q", r=2),
                        in1=mk, op=mybir.AluOpType.add)
                    am2 = sp.tile([128, 2 * G], dt.float32, tag="am2")
                    nc.vector.tensor_reduce(
                        out=am2[:],
                        in_=alpha[:].rearrange("p (rg d) -> p rg d", d=D),
                        axis=mybir.AxisListType.X, op=mybir.AluOpType.max)
                    nam = sp.tile([128, G], dt.float32, tag="nam")
                    nc.vector.tensor_reduce(
                        out=nam[:],
                        in_=am2[:].rearrange("p (r g) -> p g r", r=2),
                        axis=mybir.AxisListType.X, op=mybir.AluOpType.max,
                        negate=True)
                    nc.vector.tensor_tensor(
                        out=alpha[:].rearrange("p (r g d) -> p r g d",
                                               r=2, g=G),
                        in0=alpha[:].rearrange("p (r g d) -> p r g d",
                                               r=2, g=G),
                        in1=nam[:].unsqueeze(1).unsqueeze(3).broadcast_to(
                            [128, 2, G, D]),
                        op=mybir.AluOpType.add)
                    ex = sp.tile([128, S2], dt.float32, tag="ex")
                    nc.scalar.activation(
                        out=ex[:], in_=alpha[:],
                        func=mybir.ActivationFunctionType.Exp)
                    exb = sp.tile([128, S2], dt.bfloat16, tag="exb")
                    nc.vector.tensor_copy(out=exb[:], in_=ex[:])
                    den2 = sp.tile([128, 2 * G], dt.float32, tag="den2")
                    nc.vector.tensor_reduce(
                        out=den2[:],
                        in_=ex[:].rearrange("p (rg d) -> p rg d", d=D),
                        axis=mybir.AxisListType.X, op=mybir.AluOpType.add)
                    den1 = sp.tile([128, G], dt.float32, tag="den1")
                    nc.vector.tensor_reduce(
                        out=den1[:],
                        in_=den2[:].rearrange("p (r g) -> p g r", r=2),
                        axis=mybir.AxisListType.X, op=mybir.AluOpType.add)
                    rden = sp.tile([128, G], dt.float32, tag="rden")
                    nc.vector.reciprocal(out=rden[:], in_=den1[:])
                    # wv overwrites prod (hj*hi no longer needed)
                    nc.vector.tensor_tensor(
                        out=prod[:],
                        in0=gb[:, :, :, 0:H],
                        in1=exb[:].rearrange("p (r q) -> p r q", r=2)
                        .unsqueeze(3).broadcast_to([128, 2, csl, H]),
                        op=mybir.AluOpType.mult)
                    agg2 = tp.tile([128, 2, G, H], dt.float32, tag="agg2")
                    nc.vector.tensor_reduce(
                        out=agg2[:].rearrange("p r g f -> p (r g) f"),
                        in_=prod[:].rearrange("p r (g d) f -> p (r g) f d",
                                              g=G),
                        axis=mybir.AxisListType.X, op=mybir.AluOpType.add)
                    agg = tp.tile([128, G, H], dt.float32, tag="agg")
                    nc.vector.tensor_tensor(
                        out=agg[:], in0=agg2[:, 0], in1=agg2[:, 1],
                        op=mybir.AluOpType.add)
                    nc.vector.tensor_tensor(
                        out=agg[:], in0=agg[:],
                        in1=rden[:].unsqueeze(2).broadcast_to([128, G, H]),
                        op=mybir.AluOpType.mult)
                    nc.vector.tensor_tensor(
                        out=agg[:], in0=agg[:],
                        in1=brep[:].rearrange("p (l h) -> p l h",
                                              l=L_FULL)[:, li - 1]
                        .unsqueeze(1).broadcast_to([128, G, H]),
                        op=mybir.AluOpType.add)
                    hnext = tp.tile([128, G, H], dt.float32, tag="hnext")
                    nc.scalar.activation(
                        out=hnext[:], in_=agg[:],
                        func=mybir.ActivationFunctionType.Relu)

                    # tails: project 4 blocks at a time via block-diag weights
                    for t in range(0, G, 4):
                        gq = min(4, G - t)
                        tps = pt.tile([128, 128], dt.float32, space="PSUM",
                                      tag="hT")
                        nc.tensor.transpose(
                            out=tps[0:gq * H, :],
                            in_=hnext[:, t:t + gq, :].rearrange(
                                "p g f -> p (g f)"),
                            identity=ident[:])
                        hTs = sp.tile([128, 128], dt.float32, tag="hTs")
                        nc.scalar.copy(out=hTs[0:gq * H, :],
                                       in_=tps[0:gq * H, :])
                        mm = pm.tile([128, 4 * ROWW], dt.float32,
                                     space="PSUM", tag="mm")
                        if last:
                            nc.tensor.matmul(
                                out=mm[:, 0:gq * D_OUT],
                                lhsT=hTs[0:gq * H, :],
                                rhs=w16bd[0:gq * H, 0:gq * D_OUT],
                                start=True, stop=True)
                            nc.vector.tensor_tensor(
                                out=outstage[:, b0 + t:b0 + t + gq, :],
                                in0=mm[:, 0:gq * D_OUT].rearrange(
                                    "p (g w) -> p g w", g=gq),
                                in1=b16r[:].unsqueeze(1).broadcast_to(
                                    [128, gq, D_OUT]),
                                op=mybir.AluOpType.add)
                        else:
                            wslice = wbd[:].rearrange(
                                "p (l w) -> p l w", l=L_FULL)[:, li - 1, :]
                            nc.tensor.matmul(
                                out=mm[:, 0:gq * ROWW],
                                lhsT=hTs[0:gq * H, :],
                                rhs=wslice[0:gq * H, 0:gq * ROWW],
                                start=True, stop=True)
                            mmv = mm[:].rearrange("p (g w) -> p g w", g=4)
                            nc.vector.tensor_copy(
                                out=own_new[:, b0 + t:b0 + t + gq, 0:H],
                                in_=mmv[:, 0:gq, 0:H])
                            nc.scalar.copy(
                                out=ownf_new[:, b0 + t:b0 + t + gq,
                                             H // 2:H // 2 + 2],
                                in_=mmv[:, 0:gq, H:H + 2])
                if last:
                    nc.sync.dma_start(
                        out=out_d[:].rearrange("(b p) w -> p b w", p=128),
                        in_=outstage[:])
                elif li == LN:
                    nc.sync.dma_start(
                        out=out_d[:].rearrange("(b p) w -> p b w", p=128),
                        in_=own_new[:].bitcast(dt.uint16))
                else:
                    table = exchange(own_new)

    nc.compile()
    return nc


# ----------------------------------------------------------------------------
# entry point
# ----------------------------------------------------------------------------

_CACHE = {}
LAST_RES = None


def kernel(x, edge_index, W0, b0, Ws, att_l, att_r, bs, W16, b16):
    x = np.asarray(x, dtype=np.float32)
    edge_index = np.asarray(edge_index)
    pre = _preprocess(edge_index)
    wts = _prep_weights(np.asarray(W0, np.float32), np.asarray(b0, np.float32),
                        np.asarray(Ws, np.float32),
                        np.asarray(att_l, np.float32),
                        np.asarray(att_r, np.float32),
                        np.asarray(bs, np.float32),
                        np.asarray(W16, np.float32),
                        np.asarray(b16, np.float32))
    key = pre["key"]
    if key not in _CACHE:
        _CACHE[key] = _build_program(pre["sched"])
    nc = _CACHE[key]

    inv_perm = pre["inv_perm"]
    in_maps = []
    for c in range(NC):
        pids = np.arange(c * NPC, (c + 1) * NPC)
        orig = inv_perm[pids]
        xT = np.zeros((D_IN, NPAD), np.float32)
        xT[:, 0:NPC] = x[orig].T
        in_maps.append(dict(
            xT=xT, idxA=pre["idxA"][c], idxB=pre["idxB"][c],
            mask=pre["mask"][c].reshape(128, -1),
            wfold=wts["wfold"], bfold=wts["bfold"], wbd=wts["wbd"],
            w16bd=wts["w16bd"], brep=wts["brep"], b16rep=wts["b16rep"]))

    trace = bool(int(os.environ.get("SGAT_TRACE", "0")))
    tdir = os.environ.get("SGAT_TRACE_DIR") or None
    res = bass_utils.run_bass_kernel_spmd(nc, in_maps, core_ids=list(range(NC)),
                                          trace=trace, tmpdir=tdir)
    global LAST_RES
    LAST_RES = res

    if L_DEBUG >= L_FULL:
        out = np.zeros((N, D_OUT), np.float32)
        for c in range(NC):
            pids = np.arange(c * NPC, (c + 1) * NPC)
            out[inv_perm[pids]] = res.results[c]["out"][0:NPC]
        return out
    else:
        # debug: return raw table_{L+1} rows per permuted id
        out = np.zeros((N, ROWW), np.uint16)
        for c in range(NC):
            pids = np.arange(c * NPC, (c + 1) * NPC)
            out[inv_perm[pids]] = res.results[c]["out"][0:NPC]
        return out


# revision 16
# speedup vs baseline: 1.2410x; 1.1288x over previous
"""SuperGAT x15 Trainium2 kernel (8 NeuronCores, SPMD).

Self-contained: hardcodes all shapes. Strategy:
- Nodes permuted by "need" (balanced split slots), striped across 8 cores
  (core = rank % 8, pos = rank // 8). Each core owns 6250 nodes and all
  edges whose dst it owns.
- Per layer, each core holds a replicated DRAM table of rows
  [hp(32) bf16 | aL f32 | aR f32] = 36 bf16-slots = 72B at 256B stride.
- Messages gathered per edge-slot via dma_gather (int16 idxs) round-robin
  over 4 SWDGE queues with lookahead. The int16 range limit (32767) is
  handled with two overlapping table views: region A = rows [0, 32768),
  region B = rows [17232, 50000). Each node's in-edges are split between
  regions, padded to a UNIFORM per-chunk slot count D (SPMD: one program).
- Layout: node-per-partition, slots along free axis. Whole chunks (G
  blocks x D slots) are processed by single wide vector ops; segment
  softmax = free-axis reductions with 4D access patterns.
- Tails (h @ W_aug projection for the next layer) batched 4 blocks per
  matmul using block-diagonal weights.
- Per-layer exchange: own table rows -> DRAM bounce -> AllGather ->
  spread DMA into the 256B-stride gather table.
"""
import os
import hashlib
import numpy as np
import ml_dtypes

import concourse.bacc as bacc
import concourse.bass as bass
import concourse.tile as tile
from concourse import mybir, bass_utils, library_config
from concourse.masks import make_identity

dt = mybir.dt

# problem constants
N = 50000
E = 800000
D_IN = 128
H = 32
D_OUT = 16
L_FULL = 15
NEG = 0.2
NC = 8
NPC = N // NC            # 6250 nodes per core
NBLK = (NPC + 127) // 128  # 49 blocks
NPAD = NBLK * 128        # 6272 padded positions
T_HI = 32768
T_LO = N - T_HI          # 17232
ROWW = 36                # bf16 slots per table row (72B payload)
TABW = 128               # bf16 slots per table row stride (256B)

L_DEBUG = int(os.environ.get("SGAT_LAYERS", str(L_FULL)))
MAX_IDX_PER_GATHER = 16000
CHUNK_SLOTS = int(os.environ.get("SGAT_CHUNK", "80"))  # per-partition per-region
NSWQ = int(os.environ.get("SGAT_NSWQ", "4"))
LOOK = int(os.environ.get("SGAT_LOOK", "1"))


def _patch_dma_gather_assert():
    import inspect, textwrap
    if getattr(bass.BassGpSimd.dma_gather, "_sgat_patched", False):
        return
    src = inspect.getsource(bass.BassGpSimd.dma_gather)
    src = src.replace(
        "assert (\n            elem_size_bytes > 0 and elem_size_bytes % 256 == 0\n        )  # transpose restriction",
        "assert elem_size_bytes > 0")
    src = textwrap.dedent(src)
    ns = dict(bass.BassGpSimd.dma_gather.__globals__)
    exec(src, ns)
    fn = ns["dma_gather"]
    fn._sgat_patched = True
    bass.BassGpSimd.dma_gather = fn


_patch_dma_gather_assert()


# ----------------------------------------------------------------------------
# host-side graph preprocessing
# ----------------------------------------------------------------------------

def _preprocess(edge_index):
    src0 = edge_index[0].astype(np.int64)
    dst0 = edge_index[1].astype(np.int64)
    loops = np.arange(N, dtype=np.int64)
    src0 = np.concatenate([src0, loops])
    dst0 = np.concatenate([dst0, loops])

    deg = np.bincount(dst0, minlength=N)
    # Two-pass permutation: sort by degree, compute per-node "need"
    # (slots per region), re-sort by need so block maxima are tight.
    r = np.arange(N, dtype=np.int64)
    pid_of_rank = (r % NC) * NPC + r // NC

    def mk_perm(key):
        rank_of = np.argsort(-key, kind="stable")
        perm = np.empty(N, dtype=np.int64)
        perm[rank_of] = pid_of_rank
        return perm

    def calc_need(perm):
        psrc = perm[src0]
        pdst = perm[dst0]
        pdeg = np.bincount(pdst, minlength=N)
        nAf = np.bincount(pdst[psrc < T_LO], minlength=N)
        nBf = np.bincount(pdst[psrc >= T_HI], minlength=N)
        need = np.maximum(np.maximum(nAf, nBf), (pdeg + 1) // 2)
        return need, need[perm]

    perm = mk_perm(deg)
    _, need_orig = calc_need(perm)
    perm = mk_perm(need_orig)
    need, _ = calc_need(perm)
    inv_perm = np.empty(N, dtype=np.int64)
    inv_perm[perm] = np.arange(N, dtype=np.int64)

    psrc = perm[src0]
    pdst = perm[dst0]

    # block schedule: Dh[b] = max need over all cores' block b
    need_pad = np.zeros(NC * NPAD, dtype=np.int64)
    node_pid = np.arange(N)
    need_pad[(node_pid // NPC) * NPAD + node_pid % NPC] = need
    Dh = need_pad.reshape(NC, NBLK, 128).max(axis=(0, 2)).astype(np.int64)
    Dh = np.maximum(Dh, 1)

    # chunks of G consecutive blocks sharing a uniform slot count D.
    # First chunks are kept small so the first gather of each layer drains
    # quickly and vector work starts early (pipeline warm-up).
    chunks = []  # (b0, G, D, q0)
    offq = np.zeros(NBLK, dtype=np.int64)
    Dcap = np.zeros(NBLK, dtype=np.int64)
    b = 0
    q = 0
    while b < NBLK:
        cap = (40 if len(chunks) == 0 else
               64 if len(chunks) == 1 else CHUNK_SLOTS)
        d = int(Dh[b])
        g = 1
        while b + g < NBLK:
            nd = max(d, int(Dh[b + g]))
            if (g + 1) * nd > cap or (g + 1) * nd * 128 > MAX_IDX_PER_GATHER:
                break
            d = nd
            g += 1
        chunks.append((b, g, d, q))
        for j in range(g):
            offq[b + j] = q + j * d
            Dcap[b + j] = d
        q += g * d
        b += g
    SA = int(q)

    # per-core slot tables
    eorder = np.lexsort((psrc, pdst))
    s_src = psrc[eorder]
    s_dst = pdst[eorder]
    starts = np.searchsorted(s_dst, np.arange(N))
    ends = np.searchsorted(s_dst, np.arange(N) + 1)

    idxA = np.zeros((NC, 128, SA), dtype=np.int16)
    idxB = np.zeros((NC, 128, SA), dtype=np.int16)
    maskA = np.full((NC, 128, SA), -1e30, dtype=np.float32)
    maskB = np.full((NC, 128, SA), -1e30, dtype=np.float32)

    for n in range(N):
        e0, e1 = starts[n], ends[n]
        if e0 == e1:
            continue
        ss = s_src[e0:e1]
        c = n // NPC
        p = n % NPC
        bb = p // 128
        pp = p % 128
        d = int(Dcap[bb])
        q0 = int(offq[bb])
        fa = ss[ss < T_LO]
        fb = ss[ss >= T_HI]
        fx = ss[(ss >= T_LO) & (ss < T_HI)]
        na, nb, nd = len(fa), len(fb), len(ss)
        lo_t = max(na, nd - d)
        hi_t = min(na + len(fx), d)
        ta = min(max((nd + 1) // 2, lo_t), hi_t)
        a_list = np.concatenate([fa, fx[: ta - na]])
        b_list = np.concatenate([fb, fx[ta - na:]])
        la, lb = len(a_list), len(b_list)
        assert la <= d and lb <= d, (n, la, lb, d)
        idxA[c, pp, q0:q0 + la] = a_list.astype(np.int16)
        maskA[c, pp, q0:q0 + la] = 0.0
        idxB[c, pp, q0:q0 + lb] = (b_list - T_LO).astype(np.int16)
        maskB[c, pp, q0:q0 + lb] = 0.0

    # wrap idxs for dma_gather: position i = q*128 + p -> [i%16, i//16], x8
    def wrap(idx):  # [128, SA] -> [128, SA*8] int16
        flat = idx.transpose(1, 0).reshape(-1)          # i-major
        w16 = flat.reshape(-1, 16).T                    # [16, SA*8]
        return np.tile(w16, (8, 1)).astype(np.int16)

    idxA_w = np.stack([wrap(idxA[c]) for c in range(NC)])
    idxB_w = np.stack([wrap(idxB[c]) for c in range(NC)])
    mask = np.stack([np.concatenate([maskA[c], maskB[c]], axis=1)
                     for c in range(NC)])               # [NC, 128, 2*SA]

    sched = dict(chunks=chunks, SA=SA)
    key = hashlib.sha256(
        (str(chunks) + str(L_DEBUG) + str(CHUNK_SLOTS) + str(NSWQ)
         + str(LOOK)).encode()).hexdigest()[:16]
    return dict(perm=perm, inv_perm=inv_perm, sched=sched, key=key,
                idxA=idxA_w, idxB=idxB_w, mask=mask)


# ----------------------------------------------------------------------------
# weights preprocessing
# ----------------------------------------------------------------------------

def _prep_weights(W0, b0, Ws, att_l, att_r, bs, W16, b16):
    # table_1 = (x @ W0 + b0) @ W1aug ; W1aug = [W1 | W1@al1 | W1@ar1]
    def aug(Wl, al, ar):
        A = np.zeros((H, ROWW), np.float32)
        A[:, :H] = Wl
        A[:, H] = Wl @ al
        A[:, H + 1] = Wl @ ar
        return A

    W1aug = aug(Ws[0], att_l[0], att_r[0])
    wfold = (W0 @ W1aug).astype(np.float32)            # [128, 36]
    bfold = (b0 @ W1aug).astype(np.float32)            # [36]

    # block-diagonal (4x) aug weights: wbd[li-1] is used by layer li's
    # tails to produce table_{li+1}; layer 15 uses w16bd instead.
    wbd = np.zeros((L_FULL, 128, 4 * ROWW), np.float32)
    for li in range(1, L_FULL):
        A = aug(Ws[li], att_l[li], att_r[li])
        for g in range(4):
            wbd[li - 1, g * H:(g + 1) * H, g * ROWW:(g + 1) * ROWW] = A
    w16bd = np.zeros((128, 4 * D_OUT), np.float32)
    for g in range(4):
        w16bd[g * H:(g + 1) * H, g * D_OUT:(g + 1) * D_OUT] = W16

    brep = np.tile(bs[:, None, :], (1, 128, 1)).astype(np.float32)
    bfold_rep = np.tile(bfold[None, :], (128, 1)).astype(np.float32)
    b16rep = np.tile(b16[None, :], (128, 1)).astype(np.float32)
    return dict(wfold=wfold, bfold=bfold_rep, wbd=wbd, w16bd=w16bd,
                brep=brep, b16rep=b16rep)


# ----------------------------------------------------------------------------
# program builder
# ----------------------------------------------------------------------------

def _build_program(sched):
    chunks = sched["chunks"]
    SA = sched["SA"]
    LN = L_DEBUG
    nch = len(chunks)

    nc = bacc.Bacc(num_devices=NC, num_swdge_queues=NSWQ)
    xT_in = nc.dram_tensor("xT", [D_IN, NPAD], dt.float32, kind="ExternalInput")
    idxA_in = nc.dram_tensor("idxA", [128, SA * 8], dt.int16, kind="ExternalInput")
    idxB_in = nc.dram_tensor("idxB", [128, SA * 8], dt.int16, kind="ExternalInput")
    mask_in = nc.dram_tensor("mask", [128, 2 * SA], dt.float32, kind="ExternalInput")
    wfold_in = nc.dram_tensor("wfold", [D_IN, ROWW], dt.float32, kind="ExternalInput")
    bfold_in = nc.dram_tensor("bfold", [128, ROWW], dt.float32, kind="ExternalInput")
    wbd_in = nc.dram_tensor("wbd", [L_FULL, 128, 4 * ROWW], dt.float32,
                            kind="ExternalInput")
    w16bd_in = nc.dram_tensor("w16bd", [128, 4 * D_OUT], dt.float32,
                              kind="ExternalInput")
    brep_in = nc.dram_tensor("brep", [L_FULL, 128, H], dt.float32,
                             kind="ExternalInput")
    b16_in = nc.dram_tensor("b16rep", [128, D_OUT], dt.float32,
                            kind="ExternalInput")

    if LN >= L_FULL:
        out_d = nc.dram_tensor("out", [NPAD, D_OUT], dt.float32,
                               kind="ExternalOutput")
    else:
        out_d = nc.dram_tensor("out", [NPAD, ROWW], dt.uint16,
                               kind="ExternalOutput")

    with tile.TileContext(nc) as tc:
        with tc.tile_pool(name="res", bufs=1) as res, \
             tc.tile_pool(name="gp", bufs=LOOK + 2) as gp, \
             tc.tile_pool(name="bp", bufs=2) as bp, \
             tc.tile_pool(name="wp", bufs=2) as wp, \
             tc.tile_pool(name="sp", bufs=2) as sp, \
             tc.tile_pool(name="tp", bufs=2) as tp, \
             tc.tile_pool(name="xp", bufs=2) as xp, \
             tc.tile_pool(name="pt", bufs=2, space="PSUM") as pt, \
             tc.tile_pool(name="pm", bufs=2, space="PSUM") as pm, \
             tc.tile_pool(name="dram", bufs=2, space="DRAM") as dram:

            nc.gpsimd.load_library(library_config.mlp)

            # residents
            idxA = res.tile([128, SA * 8], dt.int16)
            nc.sync.dma_start(out=idxA[:], in_=idxA_in[:])
            idxB = res.tile([128, SA * 8], dt.int16)
            nc.sync.dma_start(out=idxB[:], in_=idxB_in[:])
            maskr = res.tile([128, 2 * SA], dt.float32)
            nc.sync.dma_start(out=maskr[:], in_=mask_in[:])
            wfold = res.tile([D_IN, ROWW], dt.float32)
            nc.sync.dma_start(out=wfold[:], in_=wfold_in[:])
            bfold = res.tile([128, ROWW], dt.float32)
            nc.sync.dma_start(out=bfold[:], in_=bfold_in[:])
            wbd = res.tile([128, L_FULL * 4 * ROWW], dt.float32)
            nc.sync.dma_start(
                out=wbd[:].rearrange("p (l w) -> p l w", l=L_FULL),
                in_=wbd_in[:].rearrange("l p w -> p l w"))
            w16bd = res.tile([128, 4 * D_OUT], dt.float32)
            nc.sync.dma_start(out=w16bd[:], in_=w16bd_in[:])
            brep = res.tile([128, L_FULL * H], dt.float32)
            nc.sync.dma_start(
                out=brep[:].rearrange("p (l h) -> p l h", l=L_FULL),
                in_=brep_in[:].rearrange("l p h -> p l h"))
            b16r = res.tile([128, D_OUT], dt.float32)
            nc.sync.dma_start(out=b16r[:], in_=b16_in[:])
            ident = res.tile([128, 128], dt.float32)
            make_identity(nc, ident[:])

            own_tabs = [res.tile([128, NBLK, ROWW], dt.bfloat16, name=f"own{i}")
                        for i in range(2)]
            outstage = res.tile([128, NBLK, D_OUT], dt.float32)

            # ---------------- conv0 + fold into table_1 -----------------
            own = own_tabs[0]
            ownf0 = own[:].bitcast(dt.float32)
            for qd in range((NBLK + 3) // 4):
                b0 = qd * 4
                qw = min(4, NBLK - b0)
                xq = xp.tile([D_IN, qw * 128], dt.float32, tag="xq")
                nc.sync.dma_start(out=xq[:],
                                  in_=xT_in[:, b0 * 128:(b0 + qw) * 128])
                mmc = pm.tile([128, 4 * ROWW], dt.float32, space="PSUM",
                              tag="mm")
                for g in range(qw):
                    nc.tensor.matmul(out=mmc[:, g * ROWW:(g + 1) * ROWW],
                                     lhsT=xq[:, g * 128:(g + 1) * 128],
                                     rhs=wfold[:], start=True, stop=True)
                ps2 = sp.tile([128, 4 * ROWW], dt.float32, tag="c0add")
                nc.vector.tensor_tensor(
                    out=ps2[:, 0:qw * ROWW].rearrange("p (g w) -> p g w", g=qw),
                    in0=mmc[:, 0:qw * ROWW].rearrange("p (g w) -> p g w", g=qw),
                    in1=bfold[:].unsqueeze(1).broadcast_to([128, qw, ROWW]),
                    op=mybir.AluOpType.add)
                ps2v = ps2[:].rearrange("p (g w) -> p g w", g=4)
                nc.vector.tensor_copy(
                    out=own[:, b0:b0 + qw, 0:H],
                    in_=ps2v[:, 0:qw, 0:H])
                ps2f = ps2[:].rearrange("p (g w) -> p g w", g=4)
                nc.scalar.copy(
                    out=ownf0[:, b0:b0 + qw, H // 2:H // 2 + 2],
                    in_=ps2f[:, 0:qw, H:H + 2])

            def exchange(own_tab):
                bounce = dram.tile([NPAD, ROWW], dt.bfloat16, tag="bounce")
                nc.sync.dma_start(
                    out=bounce[:].rearrange("(b p) w -> p b w", p=128),
                    in_=own_tab[:])
                table = dram.tile([N, TABW], dt.bfloat16, tag="table")
                if os.environ.get("SGAT_SPREAD", "1") == "1":
                    agout = dram.tile([N, ROWW], dt.bfloat16, tag="agout")
                    nc.gpsimd.collective_compute(
                        "AllGather", mybir.AluOpType.bypass,
                        replica_groups=[list(range(NC))],
                        ins=[bounce[0:NPC, :]], outs=[agout[:]])
                    # split the row-spread across both HWDGE rings
                    nh = N // 2
                    nc.sync.dma_start(out=table[0:nh, 0:ROWW],
                                      in_=agout[0:nh, :])
                    nc.scalar.dma_start(out=table[nh:N, 0:ROWW],
                                        in_=agout[nh:N, :])
                elif False:
                    nc.gpsimd.collective_compute(
                        "AllGather", mybir.AluOpType.bypass,
                        replica_groups=[list(range(NC))],
                        ins=[bounce[0:NPC, :]], outs=[table[:, 0:ROWW]])
                return table

            if LN == 0:
                nc.sync.dma_start(
                    out=out_d[:].rearrange("(b p) w -> p b w", p=128),
                    in_=own[:].bitcast(dt.uint16))
            table = exchange(own)

            gctr = [0]

            def do_gather(table, ci):
                b0, G, D, q0 = chunks[ci]
                csl = G * D
                gb = gp.tile([128, 2, csl, ROWW], dt.bfloat16, tag="gb")
                for rg in range(2):
                    tab_view = table[0:T_HI, 0:ROWW] if rg == 0 \
                        else table[T_LO:N, 0:ROWW]
                    idxr = idxA if rg == 0 else idxB
                    nidx = csl * 128
                    nc.gpsimd.dma_gather(
                        out_ap=gb[:, rg, :, :], in_ap=tab_view,
                        idxs_ap=idxr[:, q0 * 8:(q0 + csl) * 8],
                        num_idxs=nidx, num_idxs_reg=nidx,
                        elem_size=ROWW, elem_step=TABW,
                        single_packet=False,
                        queue_num=gctr[0] % NSWQ)
                    gctr[0] += 1
                return gb

            # ---------------- layers ----------------
            for li in range(1, LN + 1):
                own_prev = own_tabs[(li + 1) % 2]
                own_new = own_tabs[li % 2]
                ownf_prev = own_prev[:].bitcast(dt.float32)
                ownf_new = own_new[:].bitcast(dt.float32)
                last = (li == L_FULL)
                gbq = {}
                for j in range(min(LOOK + 1, nch)):
                    gbq[j] = do_gather(table, j)
                for ci in range(nch):
                    if ci + LOOK + 1 < nch:
                        gbq[ci + LOOK + 1] = do_gather(table, ci + LOOK + 1)
                    gb = gbq.pop(ci)
                    b0, G, D, q0 = chunks[ci]
                    csl = G * D
                    S2 = 2 * csl
                    gf32 = gb[:].bitcast(dt.float32)   # [128, 2, csl, 18]
                    hp_o = own_prev[:, b0:b0 + G, 0:H]
                    aR_o = ownf_prev[:, b0:b0 + G, H // 2 + 1]  # [p, G]

                    # pre-broadcast own rows across their D slots
                    hpb = bp.tile([128, csl, H], dt.bfloat16, tag="hpb")
                    nc.vector.tensor_copy(
                        out=hpb[:].rearrange("p (g d) f -> p g d f", g=G),
                        in_=hp_o.unsqueeze(2).broadcast_to([128, G, D, H]))
                    aRb = bp.tile([128, csl], dt.float32, tag="aRb")
                    nc.vector.tensor_copy(
                        out=aRb[:].rearrange("p (g d) -> p g d", g=G),
                        in_=aR_o.unsqueeze(2).broadcast_to([128, G, D]))

                    prod = wp.tile([128, 2, csl, H], dt.bfloat16, tag="prod")
                    nc.vector.tensor_tensor(
                        out=prod[:],
                        in0=gb[:, :, :, 0:H],
                        in1=hpb[:].unsqueeze(1).broadcast_to([128, 2, csl, H]),
                        op=mybir.AluOpType.mult)
                    logit = sp.tile([128, S2], dt.float32, tag="logit")
                    nc.vector.tensor_reduce(
                        out=logit[:],
                        in_=prod[:].rearrange("p r q f -> p (r q) f"),
                        axis=mybir.AxisListType.X, op=mybir.AluOpType.add)
                    sig = sp.tile([128, S2], dt.float32, tag="sig")
                    nc.scalar.activation(
                        out=sig[:], in_=logit[:],
                        func=mybir.ActivationFunctionType.Sigmoid)
                    alpha = sp.tile([128, S2], dt.float32, tag="alpha")
                    nc.vector.tensor_tensor(
                        out=alpha[:].rearrange("p (r q) -> p r q", r=2),
                        in0=gf32[:, :, :, H // 2],
                        in1=aRb[:].unsqueeze(1).broadcast_to([128, 2, csl]),
                        op=mybir.AluOpType.add)
                    nc.vector.tensor_tensor(out=alpha[:], in0=alpha[:],
                                            in1=sig[:],
                                            op=mybir.AluOpType.mult)
                    asc = sp.tile([128, S2], dt.float32, tag="asc")
                    nc.vector.tensor_scalar(
                        out=asc[:], in0=alpha[:], scalar1=NEG, scalar2=None,
                        op0=mybir.AluOpType.mult)
                    nc.vector.tensor_tensor(
                        out=alpha[:], in0=alpha[:], in1=asc[:],
                        op=mybir.AluOpType.max)
                    mk = maskr[:].rearrange("p (r s) -> p r s", r=2)[
                        :, :, q0:q0 + csl]
                    nc.vector.tensor_tensor(
                        out=alpha[:].rearrange("p (r q) -> p r q", r=2),
                        in0=alpha[:].rearrange("p (r q) -> p r q", r=2),
                        in1=mk, op=mybir.AluOpType.add)
                    am2 = sp.tile([128, 2 * G], dt.float32, tag="am2")
                    nc.vector.tensor_reduce(
                        out=am2[:],
                        in_=alpha[:].rearrange("p (rg d) -> p rg d", d=D),
                        axis=mybir.AxisListType.X, op=mybir.AluOpType.max)
                    nam = sp.tile([128, G], dt.float32, tag="nam")
                    nc.vector.tensor_reduce(
                        out=nam[:],
                        in_=am2[:].rearrange("p (r g) -> p g r", r=2),
                        axis=mybir.AxisListType.X, op=mybir.AluOpType.max,
                        negate=True)
                    nc.vector.tensor_tensor(
                        out=alpha[:].rearrange("p (r g d) -> p r g d",
                                               r=2, g=G),
                        in0=alpha[:].rearrange("p (r g d) -> p r g d",
                                               r=2, g=G),
                        in1=nam[:].unsqueeze(1).unsqueeze(3).broadcast_to(
                            [128, 2, G, D]),
                        op=mybir.AluOpType.add)
                    ex = sp.tile([128, S2], dt.float32, tag="ex")
                    nc.scalar.activation(
                        out=ex[:], in_=alpha[:],
                        func=mybir.ActivationFunctionType.Exp)
                    exb = sp.tile([128, S2], dt.bfloat16, tag="exb")
                    nc.vector.tensor_copy(out=exb[:], in_=ex[:])
                    den2 = sp.tile([128, 2 * G], dt.float32, tag="den2")
                    nc.vector.tensor_reduce(
                        out=den2[:],
                        in_=ex[:].rearrange("p (rg d) -> p rg d", d=D),
                        axis=mybir.AxisListType.X, op=mybir.AluOpType.add)
                    den1 = sp.tile([128, G], dt.float32, tag="den1")
                    nc.vector.tensor_reduce(
                        out=den1[:],
                        in_=den2[:].rearrange("p (r g) -> p g r", r=2),
                        axis=mybir.AxisListType.X, op=mybir.AluOpType.add)
                    rden = sp.tile([128, G], dt.float32, tag="rden")
                    nc.vector.reciprocal(out=rden[:], in_=den1[:])
                    # wv overwrites prod (hj*hi no longer needed)
                    nc.vector.tensor_tensor(
                        out=prod[:],
                        in0=gb[:, :, :, 0:H],
                        in1=exb[:].rearrange("p (r q) -> p r q", r=2)
                        .unsqueeze(3).broadcast_to([128, 2, csl, H]),
                        op=mybir.AluOpType.mult)
                    agg2 = tp.tile([128, 2, G, H], dt.float32, tag="agg2")
                    nc.vector.tensor_reduce(
                        out=agg2[:].rearrange("p r g f -> p (r g) f"),
                        in_=prod[:].rearrange("p r (g d) f -> p (r g) f d",
                                              g=G),
                        axis=mybir.AxisListType.X, op=mybir.AluOpType.add)
                    agg = tp.tile([128, G, H], dt.float32, tag="agg")
                    nc.vector.tensor_tensor(
                        out=agg[:], in0=agg2[:, 0], in1=agg2[:, 1],
                        op=mybir.AluOpType.add)
                    nc.vector.tensor_tensor(
                        out=agg[:], in0=agg[:],
                        in1=rden[:].unsqueeze(2).broadcast_to([128, G, H]),
                        op=mybir.AluOpType.mult)
                    nc.vector.tensor_tensor(
                        out=agg[:], in0=agg[:],
                        in1=brep[:].rearrange("p (l h) -> p l h",
                                              l=L_FULL)[:, li - 1]
                        .unsqueeze(1).broadcast_to([128, G, H]),
                        op=mybir.AluOpType.add)
                    hnext = tp.tile([128, G, H], dt.float32, tag="hnext")
                    nc.scalar.activation(
                        out=hnext[:], in_=agg[:],
                        func=mybir.ActivationFunctionType.Relu)

                    # tails: project 4 blocks at a time via block-diag weights
                    for t in range(0, G, 4):
                        gq = min(4, G - t)
                        tps = pt.tile([128, 128], dt.float32, space="PSUM",
                                      tag="hT")
                        nc.tensor.transpose(
                            out=tps[0:gq * H, :],
                            in_=hnext[:, t:t + gq, :].rearrange(
                                "p g f -> p (g f)"),
                            identity=ident[:])
                        hTs = sp.tile([128, 128], dt.float32, tag="hTs")
                        nc.scalar.copy(out=hTs[0:gq * H, :],
                                       in_=tps[0:gq * H, :])
                        mm = pm.tile([128, 4 * ROWW], dt.float32,
                                     space="PSUM", tag="mm")
                        if last:
                            nc.tensor.matmul(
                                out=mm[:, 0:gq * D_OUT],
                                lhsT=hTs[0:gq * H, :],
                                rhs=w16bd[0:gq * H, 0:gq * D_OUT],
                                start=True, stop=True)
                            nc.vector.tensor_tensor(
                                out=outstage[:, b0 + t:b0 + t + gq, :],
                                in0=mm[:, 0:gq * D_OUT].rearrange(
                                    "p (g w) -> p g w", g=gq),
                                in1=b16r[:].unsqueeze(1).broadcast_to(
                                    [128, gq, D_OUT]),
                                op=mybir.AluOpType.add)
                        else:
                            wslice = wbd[:].rearrange(
                                "p (l w) -> p l w", l=L_FULL)[:, li - 1, :]
                            nc.tensor.matmul(
                                out=mm[:, 0:gq * ROWW],
                                lhsT=hTs[0:gq * H, :],
                                rhs=wslice[0:gq * H, 0:gq * ROWW],
                                start=True, stop=True)
                            mmv = mm[:].rearrange("p (g w) -> p g w", g=4)
                            nc.vector.tensor_copy(
                                out=own_new[:, b0 + t:b0 + t + gq, 0:H],
                                in_=mmv[:, 0:gq, 0:H])
                            nc.scalar.copy(
                                out=ownf_new[:, b0 + t:b0 + t + gq,
                                             H // 2:H // 2 + 2],
                                in_=mmv[:, 0:gq, H:H + 2])
                if last:
                    nc.sync.dma_start(
                        out=out_d[:].rearrange("(b p) w -> p b w", p=128),
                        in_=outstage[:])
                elif li == LN:
                    nc.sync.dma_start(
                        out=out_d[:].rearrange("(b p) w -> p b w", p=128),
                        in_=own_new[:].bitcast(dt.uint16))
                else:
                    table = exchange(own_new)

    nc.compile()
    return nc


# ----------------------------------------------------------------------------
# entry point
# ----------------------------------------------------------------------------

_CACHE = {}
LAST_RES = None


def kernel(x, edge_index, W0, b0, Ws, att_l, att_r, bs, W16, b16):
    x = np.asarray(x, dtype=np.float32)
    edge_index = np.asarray(edge_index)
    pre = _preprocess(edge_index)
    wts = _prep_weights(np.asarray(W0, np.float32), np.asarray(b0, np.float32),
                        np.asarray(Ws, np.float32),
                        np.asarray(att_l, np.float32),
                        np.asarray(att_r, np.float32),
                        np.asarray(bs, np.float32),
                        np.asarray(W16, np.float32),
                        np.asarray(b16, np.float32))
    key = pre["key"]
    if key not in _CACHE:
        _CACHE[key] = _build_program(pre["sched"])
    nc = _CACHE[key]

    inv_perm = pre["inv_perm"]
    in_maps = []
    for c in range(NC):
        pids = np.arange(c * NPC, (c + 1) * NPC)
        orig = inv_perm[pids]
        xT = np.zeros((D_IN, NPAD), np.float32)
        xT[:, 0:NPC] = x[orig].T
        in_maps.append(dict(
            xT=xT, idxA=pre["idxA"][c], idxB=pre["idxB"][c],
            mask=pre["mask"][c].reshape(128, -1),
            wfold=wts["wfold"], bfold=wts["bfold"], wbd=wts["wbd"],
            w16bd=wts["w16bd"], brep=wts["brep"], b16rep=wts["b16rep"]))

    trace = bool(int(os.environ.get("SGAT_TRACE", "0")))
    tdir = os.environ.get("SGAT_TRACE_DIR") or None
    res = bass_utils.run_bass_kernel_spmd(nc, in_maps, core_ids=list(range(NC)),
                                          trace=trace, tmpdir=tdir)
    global LAST_RES
    LAST_RES = res

    if L_DEBUG >= L_FULL:
        out = np.zeros((N, D_OUT), np.float32)
        for c in range(NC):
            pids = np.arange(c * NPC, (c + 1) * NPC)
            out[inv_perm[pids]] = res.results[c]["out"][0:NPC]
        return out
    else:
        # debug: return raw table_{L+1} rows per permuted id
        out = np.zeros((N, ROWW), np.uint16)
        for c in range(NC):
            pids = np.arange(c * NPC, (c + 1) * NPC)
            out[inv_perm[pids]] = res.results[c]["out"][0:NPC]
        return out
